# revision 3
# baseline (speedup 1.0000x reference)
"""GNN message-passing layer (GSS GNNLayer) on 8 Trainium2 NeuronCores.

Math (see reference):
    Ax   = A @ x                 (sparse COO, E edges)
    pre1 = Ax @ W1.T + b1
    Axx  = A @ (Ax * x)
    pre2 = Axx @ W2.T + b2
    pre  = pre1 + pre2 ; out = elu(pre) ; return (pre, out)

Distribution: row-partition by destination node; core c owns dest rows
[c*5000, (c+1)*5000).

SpMM formulation (transposed accumulation, zero DVE work per chunk):
edges are sorted by (core, dest-block of 128, src-half, dest-row, src)
and cut into chunks of 128 slots. For chunk j the host precomputes a
narrow scatter matrix R_j[slot, dloc] = val (bf16, W_j = dest-row span
of the chunk across all cores, so the program is SPMD-uniform). On
device each chunk is one matmul accumulating into the block's
transposed PSUM tile:
    psT[:, roff_j : roff_j + W_j] += M_j.T @ R_j
with M_j [128 slots, D] bf16 = the slot source rows (stationary) and
R_j the moving operand (FD = W_j, ~60-cycle floor). A leading zero
matmul (start=True) clears the tile so all chunk matmuls accumulate
with start=False and windows may overlap freely.

Pass 1 sources are host-expanded into a sequential stream table
[128, TC*D] bf16 (no gathers at all). Pass 2 gathers H = Ax*x rows
(bf16, 256 B elems) from the AllGather'd table with int16 indices
(table split at 32768), one dma_gather per (block, half), 4 SWDGE
queues round-robin.

Finals per block: fin1 computes axT (bf16 copy), H^T = psT * x^T,
PE-transposes H to row-major and writes the h2sh shard; fin2 copies
Axx^T, runs the two dense matmuls with bf16 weights, adds bias and
applies ELU = relu(x) + exp(min(x,0)) - 1.
"""

import os
import numpy as np
import ml_dtypes

BF16 = ml_dtypes.bfloat16

N = 40000
D = 128
E = 640000
NCORES = 8
NSH = N // NCORES          # 5000 dest rows per core
P = 128
NB = (NSH + P - 1) // P    # 40 dest blocks per core (last has 8 rows)
SPLIT = 32768              # int16 gather index limit
NQ = 4                     # SWDGE queues for gathers

_cache = {}


def _preprocess(adj_row, adj_col, adj_val):
    """Sort/bucket edges; build chunk geometry + R/idx arrays.

    Returns dict with:
      geometry (hashable, SPMD-uniform): per block b lists of chunks
        (half, roff, W) with R-column offsets; nlo/nhi per block.
      per-core data: rall [NCORES,128,RC] bf16, idxlo/idxhi int16,
        slot arrays (core, part, streamchunk, col) for the pass-1
        stream (filled with x in _in_maps).
    """
    row = np.asarray(adj_row, np.int64)
    col = np.asarray(adj_col, np.int64)
    val = np.asarray(adj_val, np.float32)

    core = row // NSH
    loc = row - core * NSH
    blk = loc // P
    r = loc % P
    h = (col >= SPLIT).astype(np.int64)

    order = np.lexsort((col, r, h, blk, core))
    coreS = core[order]
    blkS = blk[order]
    rS = r[order]
    hS = h[order]
    colS = col[order]
    valS = val[order]

    # rank within (core, blk, h) bucket
    key3 = (coreS * NB + blkS) * 2 + hS
    nkey = NCORES * NB * 2
    counts = np.bincount(key3, minlength=nkey)
    gstart = np.concatenate([[0], np.cumsum(counts)[:-1]])
    pos = np.arange(len(key3)) - gstart[key3]
    jS = pos // P              # chunk index within (core, blk, h)
    pS = pos % P               # slot/partition within chunk

    cnt = counts.reshape(NCORES, NB, 2)
    nch = (-(-cnt // P)).max(axis=0)          # [NB, 2] chunks per (b, h)
    nlo = nch[:, 0].astype(int)
    nhi = nch[:, 1].astype(int)

    # per (blk, h, j) global dest-row window (min/max over cores)
    # chunk table id:
    maxj = int(nch.max())
    cid = (blkS * 2 + hS) * maxj + jS
    ncid = NB * 2 * maxj
    rmin = np.full(ncid, P, np.int64)
    rmax = np.full(ncid, -1, np.int64)
    np.minimum.at(rmin, cid, rS)
    np.maximum.at(rmax, cid, rS)

    # chunk sequence: for b: lo chunks, then hi chunks
    chunks = []          # list of (b, h, j, roff, W)
    rc0s = []
    rc = 0
    lostart = [0]
    histart = [0]
    stream_pos = {}      # (b,h,j) -> stream chunk index
    loseq = {}           # (b,j) -> global lo chunk seq
    hiseq = {}
    sc = 0
    tlo = 0
    thi = 0
    for b in range(NB):
        for j in range(nlo[b]):
            c = (b * 2 + 0) * maxj + j
            ro = int(rmin[c]) if rmax[c] >= 0 else 0
            W = int(rmax[c]) - ro + 1 if rmax[c] >= 0 else 1
            chunks.append((b, 0, j, ro, W))
            rc0s.append(rc)
            rc += W
            stream_pos[(b, 0, j)] = sc
            loseq[(b, j)] = tlo
            sc += 1
            tlo += 1
        for j in range(nhi[b]):
            c = (b * 2 + 1) * maxj + j
            ro = int(rmin[c]) if rmax[c] >= 0 else 0
            W = int(rmax[c]) - ro + 1 if rmax[c] >= 0 else 1
            chunks.append((b, 1, j, ro, W))
            rc0s.append(rc)
            rc += W
            stream_pos[(b, 1, j)] = sc
            hiseq[(b, j)] = thi
            sc += 1
            thi += 1
        lostart.append(tlo)
        histart.append(thi)
    RC = rc
    TCS = sc
    TLO = tlo
    THI = thi

    # maps for vectorized scatter
    rc0_map = np.zeros(ncid, np.int64)
    roff_map = np.zeros(ncid, np.int64)
    scpos_map = np.zeros(ncid, np.int64)
    seq_map = np.zeros(ncid, np.int64)
    for (b, hh, j, ro, W), rc0 in zip(chunks, rc0s):
        c = (b * 2 + hh) * maxj + j
        rc0_map[c] = rc0
        roff_map[c] = ro
        scpos_map[c] = stream_pos[(b, hh, j)]
        seq_map[c] = loseq[(b, j)] if hh == 0 else hiseq[(b, j)]

    # R matrices
    rall = np.zeros((NCORES, P, RC), np.float32)
    rall[coreS, pS, rc0_map[cid] + (rS - roff_map[cid])] = valS
    rall = rall.astype(BF16)

    # gather idx arrays (baseline layout: slot q -> partition q%16 (+16k
    # replicas), column q//16); int16 cols, CL = TLO*8, CH = THI*8
    CL = max(TLO * 8, 1)
    CH = max(THI * 8, 1)
    idxlo = np.zeros((NCORES, P, CL), np.int16)
    idxhi = np.zeros((NCORES, P, CH), np.int16)
    reps = 16 * np.arange(8)[None, :]
    m = hS == 0
    q = seq_map[cid[m]] * P + pS[m]
    idxlo[coreS[m][:, None], (q % 16)[:, None] + reps, (q // 16)[:, None]] = \
        colS[m].astype(np.int16)[:, None]
    m = hS == 1
    if m.any():
        q = seq_map[cid[m]] * P + pS[m]
        idxhi[coreS[m][:, None], (q % 16)[:, None] + reps, (q // 16)[:, None]] = \
            (colS[m] - SPLIT).astype(np.int16)[:, None]

    geom = []
    for b in range(NB):
        cb = [(hh, ro, W, rc0) for (bb, hh, j, ro, W), rc0 in zip(chunks, rc0s)
              if bb == b]
        geom.append(tuple(cb))

    return dict(
        geom=tuple(geom),
        nlo=tuple(int(x) for x in nlo), nhi=tuple(int(x) for x in nhi),
        lostart=tuple(lostart), histart=tuple(histart),
        RC=RC, TCS=TCS, CL=CL, CH=CH,
        rall=rall, idxlo=idxlo, idxhi=idxhi,
        # slot placement for the pass-1 stream (built in _in_maps)
        slot_core=coreS, slot_p=pS, slot_sc=scpos_map[cid], slot_col=colS,
    )


def _build(geom, nlo, nhi, lostart, histart, RC, TCS, CL, CH, reps=1):
    ABL = set(os.environ.get('ABL', '').split(','))
    import concourse.bacc as bacc
    import concourse.mybir as mybir
    import concourse.tile as tile
    from concourse.masks import make_identity

    f32 = mybir.dt.float32
    bf16 = mybir.dt.bfloat16
    i16 = mybir.dt.int16
    Alu = mybir.AluOpType
    Act = mybir.ActivationFunctionType

    nc = bacc.Bacc(None, target_bir_lowering=False, num_swdge_queues=NQ)
    stream1_d = nc.declare_dram_parameter("stream1", [P, TCS * D], bf16,
                                          isOutput=False)
    rall_d = nc.declare_dram_parameter("rall", [P, RC], bf16, isOutput=False)
    idxlo_d = nc.declare_dram_parameter("idxlo", [P, CL], i16, isOutput=False)
    idxhi_d = nc.declare_dram_parameter("idxhi", [P, CH], i16, isOutput=False)
    xt_d = nc.declare_dram_parameter("xt", [P, NSH], f32, isOutput=False)
    w1t_d = nc.declare_dram_parameter("w1t", [D, D], bf16, isOutput=False)
    w2t_d = nc.declare_dram_parameter("w2t", [D, D], bf16, isOutput=False)
    bsum_d = nc.declare_dram_parameter("bsum", [P, D], f32, isOutput=False)
    pre_o = nc.declare_dram_parameter("pre", [NSH, D], f32, isOutput=True)
    elu_o = nc.declare_dram_parameter("eluout", [NSH, D], f32, isOutput=True)
    h2sh = nc.dram_tensor("H2_shard", [NSH, D], bf16)
    h2full = nc.dram_tensor("H2_full", [N, D], bf16, addr_space="Shared")

    NROT = 8

    with tile.TileContext(nc) as tc:
        with (
            tc.tile_pool(name="const", bufs=1) as cpool,
            tc.tile_pool(name="mstr", bufs=3) as mstrp,
            tc.tile_pool(name="mglo", bufs=3) as mglop,
            tc.tile_pool(name="mghi", bufs=3) as mghip,
            tc.tile_pool(name="small", bufs=3) as smp,
            tc.tile_pool(name="psA", bufs=4, space="PSUM") as psA,
            tc.tile_pool(name="psT", bufs=2, space="PSUM") as psTp,
            tc.tile_pool(name="psD", bufs=2, space="PSUM") as psD,
        ):
            ident = cpool.tile([P, P], f32)
            make_identity(nc, ident[:])
            zt = cpool.tile([P, P], bf16)
            nc.vector.memset(zt[:], 0.0)
            rall_t = cpool.tile([P, RC], bf16)
            nc.sync.dma_start(rall_t[:], rall_d[:])
            idxlo_t = cpool.tile([P, CL], i16)
            nc.sync.dma_start(idxlo_t[:], idxlo_d[:])
            idxhi_t = cpool.tile([P, CH], i16)
            nc.sync.dma_start(idxhi_t[:], idxhi_d[:])
            xt_t = cpool.tile([P, NSH], f32)
            nc.sync.dma_start(xt_t[:], xt_d[:])
            bsum_t = cpool.tile([P, D], f32)
            nc.sync.dma_start(bsum_t[:], bsum_d[:])
            w1t_t = cpool.tile([D, D], bf16)
            nc.sync.dma_start(w1t_t[:], w1t_d[:])
            w2t_t = cpool.tile([D, D], bf16)
            nc.sync.dma_start(w2t_t[:], w2t_d[:])
            axT_all = cpool.tile([P, NSH], bf16)
            w1r, w2r, idr = [], [], []
            for k in range(NROT):
                t1 = cpool.tile([D, D], bf16, tag=f"w1r{k}")
                nc.vector.tensor_copy(t1[:], w1t_t[:])
                w1r.append(t1)
                t2 = cpool.tile([D, D], bf16, tag=f"w2r{k}")
                nc.vector.tensor_copy(t2[:], w2t_t[:])
                w2r.append(t2)
                t3 = cpool.tile([P, P], f32, tag=f"idr{k}")
                nc.vector.tensor_copy(t3[:], ident[:])
                idr.append(t3)

            # stream chunk offset per block
            scoff = [0]
            for b in range(NB):
                scoff.append(scoff[-1] + nlo[b] + nhi[b])

            qctr = [0]

            def run_once():
                def spmm_block(b, mlo_src, mhi_src):
                    """mlo_src/mhi_src: callables chunk_j -> lhsT AP."""
                    rows = min(P, NSH - b * P)
                    ps = psA.tile([P, P], f32, tag="seg")
                    ncH = nlo[b] + nhi[b]
                    if 'nomm' in ABL:
                        nc.tensor.matmul(ps[:, 0:rows], lhsT=zt[:, 0:P],
                                         rhs=zt[:, 0:rows],
                                         start=True, stop=True)
                        return ps
                    nc.tensor.matmul(ps[:, 0:rows], lhsT=zt[:, 0:P],
                                     rhs=zt[:, 0:rows],
                                     start=True, stop=False)
                    done = 0
                    for (hh, ro, W, rc0) in geom[b]:
                        if hh == 0:
                            lj = done
                            src = mlo_src(lj)
                        else:
                            src = mhi_src(done - nlo[b])
                        last = done == ncH - 1
                        nc.tensor.matmul(ps[:, ro:ro + W], lhsT=src,
                                         rhs=rall_t[:, rc0:rc0 + W],
                                         start=False, stop=last,
                                         skip_group_check=True)
                        done += 1
                    return ps

                # ---------- pass 1: sequential stream ----------
                def fin1(b, ps):
                    if 'nofin' in ABL:
                        return
                    rows = min(P, NSH - b * P)
                    axs = axT_all[:, b * P:b * P + rows]
                    nc.scalar.activation(axs, ps[:, 0:rows], Act.Copy)
                    ht = smp.tile([P, P], f32, tag="ht")
                    nc.vector.tensor_tensor(ht[:, 0:rows], ps[:, 0:rows],
                                            xt_t[:, b * P:b * P + rows],
                                            op=Alu.mult)
                    psh = psTp.tile([P, P], f32, tag="psh")
                    nc.tensor.transpose(psh[0:rows, :], ht[:, 0:rows],
                                        idr[b % NROT][:])
                    hsb = smp.tile([P, P], bf16, tag="hsb")
                    nc.vector.tensor_copy(hsb[0:rows, :], psh[0:rows, :])
                    nc.sync.dma_start(h2sh[b * P:b * P + rows, :],
                                      hsb[0:rows, :])

                for b in range(NB):
                    nchb = nlo[b] + nhi[b]
                    mt = mstrp.tile([P, nchb * D], bf16, tag="mt")
                    if 'nostream' in ABL:
                        nc.sync.dma_start(mt[:, 0:D], stream1_d[:, 0:D])
                    else:
                        nc.sync.dma_start(
                            mt[:], stream1_d[:, scoff[b] * D:scoff[b + 1] * D])
                    ps = spmm_block(
                        b,
                        lambda j, mt=mt: mt[:, j * D:(j + 1) * D],
                        lambda j, mt=mt, b=b: mt[:, (nlo[b] + j) * D:
                                                 (nlo[b] + j + 1) * D])
                    fin1(b, ps)

                # ---------- AllGather ----------
                if 'noag' not in ABL:
                    nc.gpsimd.collective_compute(
                        "AllGather", Alu.bypass,
                        replica_groups=[list(range(NCORES))],
                        ins=[h2sh[:]], outs=[h2full[:]])

                # ---------- pass 2: gathers ----------
                def fin2(b, ps):
                    if 'nofin' in ABL:
                        return
                    rows = min(P, NSH - b * P)
                    axxT = smp.tile([P, P], bf16, tag="axxT")
                    nc.scalar.activation(axxT[:, 0:rows], ps[:, 0:rows],
                                         Act.Copy)
                    pp = psD.tile([P, P], f32, tag="pp")
                    nc.tensor.matmul(pp[0:rows, :],
                                     lhsT=axT_all[:, b * P:b * P + rows],
                                     rhs=w1r[b % NROT][:],
                                     start=True, stop=False)
                    nc.tensor.matmul(pp[0:rows, :], lhsT=axxT[:, 0:rows],
                                     rhs=w2r[b % NROT][:],
                                     start=False, stop=True)
                    pre_sb = smp.tile([P, P], f32, tag="presb")
                    nc.vector.tensor_tensor(pre_sb[0:rows, :], pp[0:rows, :],
                                            bsum_t[0:rows, :], op=Alu.add)
                    nc.sync.dma_start(pre_o[b * P:b * P + rows, :],
                                      pre_sb[0:rows, :])
                    pos = smp.tile([P, P], f32, tag="pos")
                    nc.scalar.activation(pos[0:rows, :], pre_sb[0:rows, :],
                                         Act.Relu)
                    neg = smp.tile([P, P], f32, tag="neg")
                    nc.vector.tensor_scalar_min(neg[0:rows, :],
                                                pre_sb[0:rows, :], 0.0)
                    ex = smp.tile([P, P], f32, tag="ex")
                    nc.scalar.activation(ex[0:rows, :], neg[0:rows, :],
                                         Act.Exp)
                    elu = smp.tile([P, P], f32, tag="elu")
                    nc.vector.tensor_tensor(elu[0:rows, :], pos[0:rows, :],
                                            ex[0:rows, :], op=Alu.add)
                    nc.vector.tensor_scalar_add(elu[0:rows, :],
                                                elu[0:rows, :], -1.0)
                    nc.sync.dma_start(elu_o[b * P:b * P + rows, :],
                                      elu[0:rows, :])

                for b in range(NB):
                    nl, nh = nlo[b], nhi[b]
                    mlo = mglop.tile([P, max(nl, 1), D], bf16, tag="mlo")
                    if nl:
                        if 'nogather' in ABL:
                            nc.sync.dma_start(mlo[:, 0, :], h2full[0:P, :])
                        else:
                            nc.gpsimd.dma_gather(
                                out_ap=mlo[:, 0:nl, :],
                                in_ap=h2full[0:SPLIT, :],
                                idxs_ap=idxlo_t[:, lostart[b] * 8:
                                                lostart[b + 1] * 8],
                                num_idxs=nl * P, num_idxs_reg=nl * P,
                                elem_size=D, single_packet=False,
                                queue_num=qctr[0] % NQ)
                            qctr[0] += 1
                    if nh:
                        mhi = mghip.tile([P, nh, D], bf16, tag="mhi")
                        if 'nogather' in ABL:
                            nc.sync.dma_start(mhi[:, 0, :], h2full[0:P, :])
                        else:
                            nc.gpsimd.dma_gather(
                                out_ap=mhi[:],
                                in_ap=h2full[SPLIT:, :],
                                idxs_ap=idxhi_t[:, histart[b] * 8:
                                                histart[b + 1] * 8],
                                num_idxs=nh * P, num_idxs_reg=nh * P,
                                elem_size=D, single_packet=False,
                                queue_num=qctr[0] % NQ)
                            qctr[0] += 1
                    else:
                        mhi = None
                    ps = spmm_block(
                        b,
                        lambda j, mlo=mlo: mlo[:, j, :],
                        lambda j, mhi=mhi: mhi[:, j, :])
                    fin2(b, ps)

            for _ in range(reps):
                run_once()

    nc.compile()
    return nc


def _get_program(pp, reps=1):
    key = (pp["geom"], pp["nlo"], pp["nhi"], reps,
           os.environ.get("ABL", ""))
    if key not in _cache:
        _cache[key] = _build(pp["geom"], pp["nlo"], pp["nhi"],
                             pp["lostart"], pp["histart"],
                             pp["RC"], pp["TCS"], pp["CL"], pp["CH"],
                             reps=reps)
    return _cache[key]


def _in_maps(pp, features, W1, b1, W2, b2):
    feats = np.ascontiguousarray(np.asarray(features, np.float32))
    xbf = feats.astype(BF16)
    TCS = pp["TCS"]
    # pass-1 stream: [NCORES, 128, TCS, D] bf16; slot (core, p, sc) = x[col]
    stream = np.zeros((NCORES, P, TCS, D), BF16)
    stream[pp["slot_core"], pp["slot_p"], pp["slot_sc"], :] = \
        xbf[pp["slot_col"], :]
    stream = stream.reshape(NCORES, P, TCS * D)
    w1t = np.ascontiguousarray(np.asarray(W1, np.float32).T).astype(BF16)
    w2t = np.ascontiguousarray(np.asarray(W2, np.float32).T).astype(BF16)
    bsum = np.tile((np.asarray(b1, np.float32)
                    + np.asarray(b2, np.float32))[None, :], (P, 1))
    maps = []
    for c in range(NCORES):
        xt = np.ascontiguousarray(feats[c * NSH:(c + 1) * NSH].T)
        maps.append({
            "stream1": stream[c],
            "rall": pp["rall"][c],
            "idxlo": pp["idxlo"][c],
            "idxhi": pp["idxhi"][c],
            "xt": xt,
            "w1t": w1t,
            "w2t": w2t,
            "bsum": bsum,
        })
    return maps


def kernel(features, adj_row, adj_col, adj_val, W1, b1, W2, b2):
    from concourse.bass_utils import run_bass_kernel_spmd

    pp = _preprocess(adj_row, adj_col, adj_val)
    nc = _get_program(pp)
    maps = _in_maps(pp, features, W1, b1, W2, b2)
    res = run_bass_kernel_spmd(nc, maps, list(range(NCORES)))
    pre = np.concatenate([res.results[c]["pre"] for c in range(NCORES)], axis=0)
    out = np.concatenate([res.results[c]["eluout"] for c in range(NCORES)], axis=0)
    return (pre, out)


# revision 4
# speedup vs baseline: 6.9351x; 6.9351x over previous
"""GNN message-passing layer (GSS GNNLayer) on 8 Trainium2 NeuronCores.

Math (see reference):
    Ax   = A @ x                 (sparse COO, E edges)
    pre1 = Ax @ W1.T + b1
    Axx  = A @ (Ax * x)
    pre2 = Axx @ W2.T + b2
    pre  = pre1 + pre2 ; out = elu(pre) ; return (pre, out)

Distribution: row-partition by destination node; core c owns dest rows
[c*5000, (c+1)*5000).

SpMM formulation (transposed accumulation, zero DVE work per chunk):
edges are sorted by (core, dest-block of 128, src-half, dest-row, src)
and cut into chunks of 128 slots. For chunk j the host precomputes a
narrow scatter matrix R_j[slot, dloc] = val (bf16, W_j = dest-row span
of the chunk across all cores, so the program is SPMD-uniform). On
device each chunk is one matmul accumulating into the block's
transposed PSUM tile:
    psT[:, roff_j : roff_j + W_j] += M_j.T @ R_j
with M_j [128 slots, D] bf16 = the slot source rows (stationary) and
R_j the moving operand (FD = W_j, ~60-cycle floor). A leading zero
matmul (start=True) clears the tile so all chunk matmuls accumulate
with start=False and windows may overlap freely.

Pass 1 sources are host-expanded into a sequential stream table
[128, TC*D] bf16 (no gathers at all). Pass 2 gathers H = Ax*x rows
(bf16, 256 B elems) from the AllGather'd table with int16 indices
(table split at 32768), one dma_gather per (block, half), 4 SWDGE
queues round-robin.

Finals per block: fin1 computes axT (bf16 copy), H^T = psT * x^T,
PE-transposes H to row-major and writes the h2sh shard; fin2 copies
Axx^T, runs the two dense matmuls with bf16 weights, adds bias and
applies ELU = relu(x) + exp(min(x,0)) - 1.
"""

import os
import numpy as np
import ml_dtypes

BF16 = ml_dtypes.bfloat16

N = 40000
D = 128
E = 640000
NCORES = 8
NSH = N // NCORES          # 5000 dest rows per core
P = 128
NB = (NSH + P - 1) // P    # 40 dest blocks per core (last has 8 rows)
SPLIT = 32768              # int16 gather index limit
NQ = 4                     # SWDGE queues for gathers

_cache = {}


def _preprocess(adj_row, adj_col, adj_val):
    """Sort/bucket edges; build chunk geometry + R/idx arrays.

    Returns dict with:
      geometry (hashable, SPMD-uniform): per block b lists of chunks
        (half, roff, W) with R-column offsets; nlo/nhi per block.
      per-core data: rall [NCORES,128,RC] bf16, idxlo/idxhi int16,
        slot arrays (core, part, streamchunk, col) for the pass-1
        stream (filled with x in _in_maps).
    """
    row = np.asarray(adj_row, np.int64)
    col = np.asarray(adj_col, np.int64)
    val = np.asarray(adj_val, np.float32)

    core = row // NSH
    loc = row - core * NSH
    blk = loc // P
    r = loc % P
    h = (col >= SPLIT).astype(np.int64)

    order = np.lexsort((col, r, h, blk, core))
    coreS = core[order]
    blkS = blk[order]
    rS = r[order]
    hS = h[order]
    colS = col[order]
    valS = val[order]

    # rank within (core, blk, h) bucket
    key3 = (coreS * NB + blkS) * 2 + hS
    nkey = NCORES * NB * 2
    counts = np.bincount(key3, minlength=nkey)
    gstart = np.concatenate([[0], np.cumsum(counts)[:-1]])
    pos = np.arange(len(key3)) - gstart[key3]
    jS = pos // P              # chunk index within (core, blk, h)
    pS = pos % P               # slot/partition within chunk

    cnt = counts.reshape(NCORES, NB, 2)
    nch = (-(-cnt // P)).max(axis=0)          # [NB, 2] chunks per (b, h)
    nlo = nch[:, 0].astype(int)
    nhi = nch[:, 1].astype(int)

    # per (blk, h, j) global dest-row window (min/max over cores)
    # chunk table id:
    maxj = int(nch.max())
    cid = (blkS * 2 + hS) * maxj + jS
    ncid = NB * 2 * maxj
    rmin = np.full(ncid, P, np.int64)
    rmax = np.full(ncid, -1, np.int64)
    np.minimum.at(rmin, cid, rS)
    np.maximum.at(rmax, cid, rS)

    # chunk sequence: for b: lo chunks, then hi chunks
    chunks = []          # list of (b, h, j, roff, W)
    rc0s = []
    rc = 0
    lostart = [0]
    histart = [0]
    stream_pos = {}      # (b,h,j) -> stream chunk index
    loseq = {}           # (b,j) -> global lo chunk seq
    hiseq = {}
    sc = 0
    tlo = 0
    thi = 0
    for b in range(NB):
        for j in range(nlo[b]):
            c = (b * 2 + 0) * maxj + j
            ro = int(rmin[c]) if rmax[c] >= 0 else 0
            W = int(rmax[c]) - ro + 1 if rmax[c] >= 0 else 1
            chunks.append((b, 0, j, ro, W))
            rc0s.append(rc)
            rc += W
            stream_pos[(b, 0, j)] = sc
            loseq[(b, j)] = tlo
            sc += 1
            tlo += 1
        for j in range(nhi[b]):
            c = (b * 2 + 1) * maxj + j
            ro = int(rmin[c]) if rmax[c] >= 0 else 0
            W = int(rmax[c]) - ro + 1 if rmax[c] >= 0 else 1
            chunks.append((b, 1, j, ro, W))
            rc0s.append(rc)
            rc += W
            stream_pos[(b, 1, j)] = sc
            hiseq[(b, j)] = thi
            sc += 1
            thi += 1
        lostart.append(tlo)
        histart.append(thi)
    RC = rc
    TCS = sc
    TLO = tlo
    THI = thi

    # maps for vectorized scatter
    rc0_map = np.zeros(ncid, np.int64)
    roff_map = np.zeros(ncid, np.int64)
    scpos_map = np.zeros(ncid, np.int64)
    seq_map = np.zeros(ncid, np.int64)
    for (b, hh, j, ro, W), rc0 in zip(chunks, rc0s):
        c = (b * 2 + hh) * maxj + j
        rc0_map[c] = rc0
        roff_map[c] = ro
        scpos_map[c] = stream_pos[(b, hh, j)]
        seq_map[c] = loseq[(b, j)] if hh == 0 else hiseq[(b, j)]

    # R matrices
    rall = np.zeros((NCORES, P, RC), np.float32)
    rall[coreS, pS, rc0_map[cid] + (rS - roff_map[cid])] = valS
    rall = rall.astype(BF16)

    # gather idx arrays (baseline layout: slot q -> partition q%16 (+16k
    # replicas), column q//16); int16 cols, CL = TLO*8, CH = THI*8
    CL = max(TLO * 8, 1)
    CH = max(THI * 8, 1)
    idxlo = np.zeros((NCORES, P, CL), np.int16)
    idxhi = np.zeros((NCORES, P, CH), np.int16)
    reps = 16 * np.arange(8)[None, :]
    m = hS == 0
    q = seq_map[cid[m]] * P + pS[m]
    idxlo[coreS[m][:, None], (q % 16)[:, None] + reps, (q // 16)[:, None]] = \
        colS[m].astype(np.int16)[:, None]
    m = hS == 1
    if m.any():
        q = seq_map[cid[m]] * P + pS[m]
        idxhi[coreS[m][:, None], (q % 16)[:, None] + reps, (q // 16)[:, None]] = \
            (colS[m] - SPLIT).astype(np.int16)[:, None]

    geom = []
    for b in range(NB):
        cb = [(hh, ro, W, rc0) for (bb, hh, j, ro, W), rc0 in zip(chunks, rc0s)
              if bb == b]
        geom.append(tuple(cb))

    return dict(
        geom=tuple(geom),
        nlo=tuple(int(x) for x in nlo), nhi=tuple(int(x) for x in nhi),
        lostart=tuple(lostart), histart=tuple(histart),
        RC=RC, TCS=TCS, CL=CL, CH=CH,
        rall=rall, idxlo=idxlo, idxhi=idxhi,
        # slot placement for the pass-1 stream (built in _in_maps)
        slot_core=coreS, slot_p=pS, slot_sc=scpos_map[cid], slot_col=colS,
    )


def _build(geom, nlo, nhi, lostart, histart, RC, TCS, CL, CH, reps=1):
    ABL = set(os.environ.get('ABL', '').split(','))
    import concourse.bacc as bacc
    import concourse.mybir as mybir
    import concourse.tile as tile
    from concourse.masks import make_identity

    f32 = mybir.dt.float32
    bf16 = mybir.dt.bfloat16
    i16 = mybir.dt.int16
    Alu = mybir.AluOpType
    Act = mybir.ActivationFunctionType

    nc = bacc.Bacc(None, target_bir_lowering=False, num_swdge_queues=NQ,
                   dynamic_dma_scratch_size=int(os.environ.get("DDS", 65536)))
    stream1_d = nc.declare_dram_parameter("stream1", [P, TCS * D], bf16,
                                          isOutput=False)
    rall_d = nc.declare_dram_parameter("rall", [P, RC], bf16, isOutput=False)
    idxlo_d = nc.declare_dram_parameter("idxlo", [P, CL], i16, isOutput=False)
    idxhi_d = nc.declare_dram_parameter("idxhi", [P, CH], i16, isOutput=False)
    xt_d = nc.declare_dram_parameter("xt", [P, NSH], f32, isOutput=False)
    w1t_d = nc.declare_dram_parameter("w1t", [D, D], bf16, isOutput=False)
    w2t_d = nc.declare_dram_parameter("w2t", [D, D], bf16, isOutput=False)
    bsum_d = nc.declare_dram_parameter("bsum", [P, D], f32, isOutput=False)
    pre_o = nc.declare_dram_parameter("pre", [NSH, D], f32, isOutput=True)
    elu_o = nc.declare_dram_parameter("eluout", [NSH, D], f32, isOutput=True)
    h2sh = nc.dram_tensor("H2_shard", [NSH, D], bf16)
    h2full = nc.dram_tensor("H2_full", [N, D], bf16, addr_space="Shared")

    NROT = 8

    with tile.TileContext(nc) as tc:
        with (
            tc.tile_pool(name="const", bufs=1) as cpool,
            tc.tile_pool(name="mstr", bufs=3) as mstrp,
            tc.tile_pool(name="mglo", bufs=3) as mglop,
            tc.tile_pool(name="mghi", bufs=3) as mghip,
            tc.tile_pool(name="small", bufs=3) as smp,
            tc.tile_pool(name="psA", bufs=4, space="PSUM") as psA,
            tc.tile_pool(name="psT", bufs=2, space="PSUM") as psTp,
            tc.tile_pool(name="psD", bufs=2, space="PSUM") as psD,
        ):
            ident = cpool.tile([P, P], f32)
            make_identity(nc, ident[:])
            zt = cpool.tile([P, P], bf16)
            nc.vector.memset(zt[:], 0.0)
            rall_t = cpool.tile([P, RC], bf16)
            nc.sync.dma_start(rall_t[:], rall_d[:])
            idxlo_t = cpool.tile([P, CL], i16)
            nc.sync.dma_start(idxlo_t[:], idxlo_d[:])
            idxhi_t = cpool.tile([P, CH], i16)
            nc.sync.dma_start(idxhi_t[:], idxhi_d[:])
            xt_t = cpool.tile([P, NSH], f32)
            nc.sync.dma_start(xt_t[:], xt_d[:])
            bsum_t = cpool.tile([P, D], f32)
            nc.sync.dma_start(bsum_t[:], bsum_d[:])
            w1t_t = cpool.tile([D, D], bf16)
            nc.sync.dma_start(w1t_t[:], w1t_d[:])
            w2t_t = cpool.tile([D, D], bf16)
            nc.sync.dma_start(w2t_t[:], w2t_d[:])
            axT_all = cpool.tile([P, NSH], bf16)
            w1r, w2r, idr = [], [], []
            for k in range(NROT):
                t1 = cpool.tile([D, D], bf16, tag=f"w1r{k}")
                nc.vector.tensor_copy(t1[:], w1t_t[:])
                w1r.append(t1)
                t2 = cpool.tile([D, D], bf16, tag=f"w2r{k}")
                nc.vector.tensor_copy(t2[:], w2t_t[:])
                w2r.append(t2)
                t3 = cpool.tile([P, P], f32, tag=f"idr{k}")
                nc.vector.tensor_copy(t3[:], ident[:])
                idr.append(t3)

            # stream chunk offset per block
            scoff = [0]
            for b in range(NB):
                scoff.append(scoff[-1] + nlo[b] + nhi[b])

            qctr = [0]

            def run_once():
                def spmm_block(b, mlo_src, mhi_src):
                    """mlo_src/mhi_src: callables chunk_j -> lhsT AP."""
                    rows = min(P, NSH - b * P)
                    ps = psA.tile([P, P], f32, tag="seg")
                    ncH = nlo[b] + nhi[b]
                    if 'nomm' in ABL:
                        nc.tensor.matmul(ps[:, 0:rows], lhsT=zt[:, 0:P],
                                         rhs=zt[:, 0:rows],
                                         start=True, stop=True)
                        return ps
                    nc.tensor.matmul(ps[:, 0:rows], lhsT=zt[:, 0:P],
                                     rhs=zt[:, 0:rows],
                                     start=True, stop=False)
                    done = 0
                    for (hh, ro, W, rc0) in geom[b]:
                        if hh == 0:
                            lj = done
                            src = mlo_src(lj)
                        else:
                            src = mhi_src(done - nlo[b])
                        last = done == ncH - 1
                        nc.tensor.matmul(ps[:, ro:ro + W], lhsT=src,
                                         rhs=rall_t[:, rc0:rc0 + W],
                                         start=False, stop=last,
                                         skip_group_check=True)
                        done += 1
                    return ps

                # ---------- pass 1: sequential stream ----------
                def fin1(b, ps):
                    if 'nofin' in ABL:
                        return
                    rows = min(P, NSH - b * P)
                    axs = axT_all[:, b * P:b * P + rows]
                    nc.scalar.activation(axs, ps[:, 0:rows], Act.Copy)
                    ht = smp.tile([P, P], f32, tag="ht")
                    nc.vector.tensor_tensor(ht[:, 0:rows], ps[:, 0:rows],
                                            xt_t[:, b * P:b * P + rows],
                                            op=Alu.mult)
                    psh = psTp.tile([P, P], f32, tag="psh")
                    nc.tensor.transpose(psh[0:rows, :], ht[:, 0:rows],
                                        idr[b % NROT][:])
                    hsb = smp.tile([P, P], bf16, tag="hsb")
                    nc.vector.tensor_copy(hsb[0:rows, :], psh[0:rows, :])
                    nc.sync.dma_start(h2sh[b * P:b * P + rows, :],
                                      hsb[0:rows, :])

                for b in range(NB):
                    nchb = nlo[b] + nhi[b]
                    mt = mstrp.tile([P, nchb * D], bf16, tag="mt")
                    if 'nostream' in ABL:
                        nc.sync.dma_start(mt[:, 0:D], stream1_d[:, 0:D])
                    else:
                        nc.sync.dma_start(
                            mt[:], stream1_d[:, scoff[b] * D:scoff[b + 1] * D])
                    ps = spmm_block(
                        b,
                        lambda j, mt=mt: mt[:, j * D:(j + 1) * D],
                        lambda j, mt=mt, b=b: mt[:, (nlo[b] + j) * D:
                                                 (nlo[b] + j + 1) * D])
                    fin1(b, ps)

                # ---------- AllGather ----------
                if 'noag' not in ABL:
                    nc.gpsimd.collective_compute(
                        "AllGather", Alu.bypass,
                        replica_groups=[list(range(NCORES))],
                        ins=[h2sh[:]], outs=[h2full[:]])

                # ---------- pass 2: gathers ----------
                def fin2(b, ps):
                    if 'nofin' in ABL:
                        return
                    rows = min(P, NSH - b * P)
                    axxT = smp.tile([P, P], bf16, tag="axxT")
                    nc.scalar.activation(axxT[:, 0:rows], ps[:, 0:rows],
                                         Act.Copy)
                    pp = psD.tile([P, P], f32, tag="pp")
                    nc.tensor.matmul(pp[0:rows, :],
                                     lhsT=axT_all[:, b * P:b * P + rows],
                                     rhs=w1r[b % NROT][:],
                                     start=True, stop=False)
                    nc.tensor.matmul(pp[0:rows, :], lhsT=axxT[:, 0:rows],
                                     rhs=w2r[b % NROT][:],
                                     start=False, stop=True)
                    pre_sb = smp.tile([P, P], f32, tag="presb")
                    nc.vector.tensor_tensor(pre_sb[0:rows, :], pp[0:rows, :],
                                            bsum_t[0:rows, :], op=Alu.add)
                    nc.sync.dma_start(pre_o[b * P:b * P + rows, :],
                                      pre_sb[0:rows, :])
                    pos = smp.tile([P, P], f32, tag="pos")
                    nc.scalar.activation(pos[0:rows, :], pre_sb[0:rows, :],
                                         Act.Relu)
                    neg = smp.tile([P, P], f32, tag="neg")
                    nc.vector.tensor_scalar_min(neg[0:rows, :],
                                                pre_sb[0:rows, :], 0.0)
                    ex = smp.tile([P, P], f32, tag="ex")
                    nc.scalar.activation(ex[0:rows, :], neg[0:rows, :],
                                         Act.Exp)
                    elu = smp.tile([P, P], f32, tag="elu")
                    nc.vector.tensor_tensor(elu[0:rows, :], pos[0:rows, :],
                                            ex[0:rows, :], op=Alu.add)
                    nc.vector.tensor_scalar_add(elu[0:rows, :],
                                                elu[0:rows, :], -1.0)
                    nc.sync.dma_start(elu_o[b * P:b * P + rows, :],
                                      elu[0:rows, :])

                for b in range(NB):
                    nl, nh = nlo[b], nhi[b]
                    mlo = mglop.tile([P, max(nl, 1), D], bf16, tag="mlo")
                    if nl:
                        if 'nogather' in ABL:
                            nc.sync.dma_start(mlo[:, 0, :], h2full[0:P, :])
                        else:
                            nc.gpsimd.dma_gather(
                                out_ap=mlo[:, 0:nl, :],
                                in_ap=h2full[0:SPLIT, :],
                                idxs_ap=idxlo_t[:, lostart[b] * 8:
                                                lostart[b + 1] * 8],
                                num_idxs=nl * P, num_idxs_reg=nl * P,
                                elem_size=D, single_packet=False,
                                queue_num=qctr[0] % NQ)
                            qctr[0] += 1
                    if nh:
                        mhi = mghip.tile([P, nh, D], bf16, tag="mhi")
                        if 'nogather' in ABL:
                            nc.sync.dma_start(mhi[:, 0, :], h2full[0:P, :])
                        else:
                            nc.gpsimd.dma_gather(
                                out_ap=mhi[:],
                                in_ap=h2full[SPLIT:, :],
                                idxs_ap=idxhi_t[:, histart[b] * 8:
                                                histart[b + 1] * 8],
                                num_idxs=nh * P, num_idxs_reg=nh * P,
                                elem_size=D, single_packet=False,
                                queue_num=qctr[0] % NQ)
                            qctr[0] += 1
                    else:
                        mhi = None
                    ps = spmm_block(
                        b,
                        lambda j, mlo=mlo: mlo[:, j, :],
                        lambda j, mhi=mhi: mhi[:, j, :])
                    fin2(b, ps)

            for _ in range(reps):
                run_once()

    nc.compile()
    return nc


def _get_program(pp, reps=1):
    key = (pp["geom"], pp["nlo"], pp["nhi"], reps,
           os.environ.get("ABL", ""))
    if key not in _cache:
        _cache[key] = _build(pp["geom"], pp["nlo"], pp["nhi"],
                             pp["lostart"], pp["histart"],
                             pp["RC"], pp["TCS"], pp["CL"], pp["CH"],
                             reps=reps)
    return _cache[key]


def _in_maps(pp, features, W1, b1, W2, b2):
    feats = np.ascontiguousarray(np.asarray(features, np.float32))
    xbf = feats.astype(BF16)
    TCS = pp["TCS"]
    # pass-1 stream: [NCORES, 128, TCS, D] bf16; slot (core, p, sc) = x[col]
    stream = np.zeros((NCORES, P, TCS, D), BF16)
    stream[pp["slot_core"], pp["slot_p"], pp["slot_sc"], :] = \
        xbf[pp["slot_col"], :]
    stream = stream.reshape(NCORES, P, TCS * D)
    w1t = np.ascontiguousarray(np.asarray(W1, np.float32).T).astype(BF16)
    w2t = np.ascontiguousarray(np.asarray(W2, np.float32).T).astype(BF16)
    bsum = np.tile((np.asarray(b1, np.float32)
                    + np.asarray(b2, np.float32))[None, :], (P, 1))
    maps = []
    for c in range(NCORES):
        xt = np.ascontiguousarray(feats[c * NSH:(c + 1) * NSH].T)
        maps.append({
            "stream1": stream[c],
            "rall": pp["rall"][c],
            "idxlo": pp["idxlo"][c],
            "idxhi": pp["idxhi"][c],
            "xt": xt,
            "w1t": w1t,
            "w2t": w2t,
            "bsum": bsum,
        })
    return maps


def kernel(features, adj_row, adj_col, adj_val, W1, b1, W2, b2):
    from concourse.bass_utils import run_bass_kernel_spmd

    pp = _preprocess(adj_row, adj_col, adj_val)
    nc = _get_program(pp)
    maps = _in_maps(pp, features, W1, b1, W2, b2)
    res = run_bass_kernel_spmd(nc, maps, list(range(NCORES)))
    pre = np.concatenate([res.results[c]["pre"] for c in range(NCORES)], axis=0)
    out = np.concatenate([res.results[c]["eluout"] for c in range(NCORES)], axis=0)
    return (pre, out)
